# revision 15
# baseline (speedup 1.0000x reference)
"""Trainium2 Bass kernel for nn_DecoderLayer (GQA attention + top-2-of-8 MoE).

Sharding (8 NeuronCores, SPMD single program):
  - Attention: data-parallel. Core c owns batch c//2 and an interleaved set of
    512 query rows (128-token blocks 2j + c%2, j=0..3) so the causal-length
    structure is identical on every core; per-core mask tiles carry the
    difference as data.
  - MoE: expert-parallel, dense. Core c owns expert c; an AllGather shares the
    post-attention normed activations, every core runs its expert over all
    4096 tokens scaled by the top-2 combine weight, and a ReduceScatter(add)
    returns each core its own tokens' sum over experts.
  - All activations are kept feature-major ([feature, token]) so matmul chains
    need no transposes; per-token scalars are broadcast across partitions with
    K=1 matmuls (or DMA broadcast from DRAM).
  - w_ln1/w_ln2, the 1/sqrt(HD) score scale, and the "own expert = column 0"
    Wgate permutation are folded into the weights on the host.
  - Matmuls run as float32r (FP22, 1-pass PE).

kernel(**inputs) takes the full unsharded inputs and returns the full output.
"""

import sys
from contextlib import ExitStack

if "/opt/trn_rl_repo" not in sys.path:
    sys.path.insert(0, "/opt/trn_rl_repo")

import ml_dtypes
import numpy as np

import concourse.bass as bass  # noqa: F401
import concourse.mybir as mybir
import concourse.tile as tile
from concourse import bacc
from concourse.bass_utils import run_bass_kernel_spmd

P = 128
B, S, H = 4, 1024, 1024
NH, NKV, HD = 16, 4, 64
E, F = 8, 2048
EPS = 1e-6
N_CORES = 8
R = 512                      # tokens owned per core
HC = H // P                  # 8 feature chunks of H
FC = F // P                  # 16 feature chunks of F
f32 = mybir.dt.float32
f32r = mybir.dt.float32r
bf16 = mybir.dt.bfloat16
AF = mybir.ActivationFunctionType
OP = mybir.AluOpType
AX = mybir.AxisListType


def _r(ap):
    """Matmul operands are float32r-typed tiles already; identity."""
    return ap


def _jmin(kvb):
    # first local q-chunk whose kv extent (2j+2 blocks) includes kv block kvb
    return max(0, -(-(kvb - 1) // 2))


def build_program():
    nc = bacc.Bacc("TRN2", target_bir_lowering=False, debug=False,
                   num_devices=N_CORES)

    xt = nc.declare_dram_parameter("xt", [HC, P, S], f32, isOutput=False)
    xot = nc.declare_dram_parameter("xot", [HC, P, R], f32, isOutput=False)
    maskt = nc.declare_dram_parameter("maskt", [8, P, P], f32, isOutput=False)
    wq = nc.declare_dram_parameter("wq", [4, HC, P, 256], f32, isOutput=False)
    wk = nc.declare_dram_parameter("wk", [HC, P, 256], f32, isOutput=False)
    wv = nc.declare_dram_parameter("wv", [HC, P, 256], f32, isOutput=False)
    wo = nc.declare_dram_parameter("wo", [NH, HD, H], f32, isOutput=False)
    wgate = nc.declare_dram_parameter("wgate", [HC, P, E], f32, isOutput=False)
    wg = nc.declare_dram_parameter("wg", [FC // 2, HC, P, 256], bf16, isOutput=False)
    wu = nc.declare_dram_parameter("wu", [FC // 2, HC, P, 256], bf16, isOutput=False)
    wd = nc.declare_dram_parameter("wd", [HC // 2, FC, P, 256], bf16, isOutput=False)
    out = nc.declare_dram_parameter("out", [HC, P, R], f32, isOutput=True)

    with tile.TileContext(nc) as tc:
        _emit(nc, tc, xt=xt, xot=xot, maskt=maskt, wq=wq, wk=wk, wv=wv, wo=wo,
              wgate=wgate, wg=wg, wu=wu, wd=wd, out=out)
    nc.compile()
    return nc


def _emit(nc, tc, *, xt, xot, maskt, wq, wk, wv, wo, wgate, wg, wu, wd, out):
    rg = [list(range(N_CORES))]

    with ExitStack() as top:
        const = top.enter_context(tc.tile_pool(name="const", bufs=1))
        dram = top.enter_context(tc.tile_pool(name="dram", bufs=1, space="DRAM"))

        ones_col = const.tile([P, 1], f32, tag="ones_col")   # lhsT for partition sums
        ones_row = const.tile([1, P], f32, tag="ones_row")   # lhsT for broadcasts
        eps_b = const.tile([1, 1], f32, tag="eps_b")
        ones_full = const.tile([P, 64], f32, tag="ones_full")
        nc.any.memset(ones_full[:], 1.0)
        nc.any.memset(ones_col[:], 1.0)
        nc.any.memset(ones_row[:], 1.0)
        nc.any.memset(eps_b[:], EPS)

        maskt_sb = const.tile([P, 8, P], f32, tag="maskt")
        nc.sync.dma_start(maskt_sb[:], maskt[:].rearrange("m p q -> p m q"))
        wgate_sb = const.tile([P, HC, E], f32, tag="wgate")
        nc.sync.dma_start(wgate_sb[:], wgate[:].rearrange("k p e -> p k e"))

        # DRAM scratch
        ag_in = dram.tile([HC, P, R], f32, tag="ag_in")
        ag_out = dram.tile([N_CORES, HC, P, R], f32, tag="ag_out", addr_space="Shared")
        rs_in = dram.tile([N_CORES, HC, P, R], bf16, tag="rs_in")
        rs_out = dram.tile([HC, P, R], bf16, tag="rs_out")
        c_dram = dram.tile([8, P, 4], f32, tag="c_dram")

        hown = [const.tile([P, R], f32, tag=f"hown{m}", name=f"hown{m}")
                for m in range(HC)]

        with ExitStack() as attlife:
            attp = attlife.enter_context(tc.tile_pool(name="attp", bufs=1))
            kvq_stack = attlife.enter_context(ExitStack())
            kvq = kvq_stack.enter_context(tc.tile_pool(name="kvq", bufs=1))

            # -------- Phases A-C: x load, rmsnorm1, K/V/Q projections --------
            # rmsnorm1 is applied IN PLACE: after phase A, xt_sb/xot_sb hold
            # the normalized activations (xln / xlno).
            with ExitStack() as phA:
                xp = phA.enter_context(tc.tile_pool(name="xtp", bufs=1))
                xt_sb = [xp.tile([P, S], f32, tag=f"xt{i}", name=f"xt{i}")
                         for i in range(HC)]
                xot_sb = [xp.tile([P, R], f32, tag=f"xoA{i}", name=f"xoA{i}")
                          for i in range(HC)]
                xln, xlno = xt_sb, xot_sb

                with ExitStack() as ph:
                    sqp = ph.enter_context(tc.tile_pool(name="sqp", bufs=1))
                    psA = ph.enter_context(tc.tile_pool(name="psA", bufs=1, space="PSUM"))
                    rowp = ph.enter_context(tc.tile_pool(name="rowA", bufs=1))

                    ss_ps = [psA.tile([1, R], f32, tag=f"ss{n}", name=f"ss{n}")
                             for n in range(2)]
                    sso_ps = psA.tile([1, R], f32, tag="sso")
                    for hc in range(HC):
                        nc.sync.dma_start(xt_sb[hc][:], xt[hc])
                        nc.sync.dma_start(xot_sb[hc][:], xot[hc])
                    for hc in range(HC):
                        sq = sqp.tile([P, S], f32, tag="sq")
                        nc.vector.tensor_mul(sq[:], xt_sb[hc][:], xt_sb[hc][:])
                        for n in range(2):
                            nc.tensor.matmul(
                                ss_ps[n][:], _r(ones_col[:]),
                                _r(sq[:, n * R:(n + 1) * R]),
                                start=(hc == 0), stop=(hc == HC - 1))
                        sqo = sqp.tile([P, R], f32, tag="sqo")
                        nc.vector.tensor_mul(sqo[:], xot_sb[hc][:], xot_sb[hc][:])
                        nc.tensor.matmul(sso_ps[:], _r(ones_col[:]), _r(sqo[:]),
                                         start=(hc == 0), stop=(hc == HC - 1))

                    inv_row = rowp.tile([1, S], f32, tag="inv")
                    invo_row = rowp.tile([1, R], f32, tag="invo")
                    for n in range(2):
                        nc.scalar.activation(inv_row[:, n * R:(n + 1) * R],
                                             ss_ps[n][:], AF.Sqrt,
                                             bias=eps_b[:], scale=1.0 / H)
                    with nc.allow_low_precision(reason="fp32r scale rows"):
                        nc.vector.reciprocal(inv_row[:], inv_row[:])
                    nc.scalar.activation(invo_row[:], sso_ps[:], AF.Sqrt,
                                         bias=eps_b[:], scale=1.0 / H)
                    with nc.allow_low_precision(reason="fp32r scale rows"):
                        nc.vector.reciprocal(invo_row[:], invo_row[:])

                    invb_ps = [psA.tile([P, R], f32, tag=f"invb{n}", name=f"invb{n}")
                               for n in range(2)]
                    invob_ps = psA.tile([P, R], f32, tag="invob")
                    for n in range(2):
                        nc.tensor.matmul(invb_ps[n][:], _r(ones_row[:]),
                                         _r(inv_row[:, n * R:(n + 1) * R]),
                                         start=True, stop=True)
                    nc.tensor.matmul(invob_ps[:], _r(ones_row[:]), _r(invo_row[:]),
                                     start=True, stop=True)
                    for hc in range(HC):
                        for n in range(2):
                            nc.vector.tensor_mul(xln[hc][:, n * R:(n + 1) * R],
                                                 xt_sb[hc][:, n * R:(n + 1) * R],
                                                 invb_ps[n][:])
                        nc.vector.tensor_mul(xlno[hc][:], xot_sb[hc][:],
                                             invob_ps[:])

                # K (per kv-head, [64, S]) and V (token-major, [128, 256])
                kTh = [kvq.tile([64, S], f32, tag=f"kTh{i}", name=f"kTh{i}")
                       for i in range(NKV)]
                v_sb = [kvq.tile([P, 4 * 65], f32, tag=f"v{i}", name=f"v{i}")
                        for i in range(HC)]
                with ExitStack() as ph:
                    wp = ph.enter_context(tc.tile_pool(name="wkv", bufs=1))
                    psB = ph.enter_context(tc.tile_pool(name="psB", bufs=2, space="PSUM"))
                    wk_sb = [wp.tile([P, 256], f32, tag=f"wk{i}", name=f"wk{i}")
                             for i in range(HC)]
                    wv_sb = [wp.tile([P, 256], f32, tag=f"wv{i}", name=f"wv{i}")
                             for i in range(HC)]
                    for hc in range(HC):
                        nc.sync.dma_start(wk_sb[hc][:], wk[hc])
                        nc.sync.dma_start(wv_sb[hc][:], wv[hc])
                    for hkv in range(NKV):
                        for n in range(2):
                            ps = psB.tile([64, R], f32, tag="psk")
                            for hc in range(HC):
                                nc.tensor.matmul(
                                    ps[:],
                                    _r(wk_sb[hc][:, hkv * 64:(hkv + 1) * 64]),
                                    _r(xln[hc][:, n * R:(n + 1) * R]),
                                    start=(hc == 0), stop=(hc == HC - 1))
                            nc.scalar.copy(kTh[hkv][:, n * R:(n + 1) * R], ps[:])
                    for kvc in range(HC):
                        ps = psB.tile([P, 256], f32, tag="psv")
                        for hc in range(HC):
                            nc.tensor.matmul(
                                ps[:], _r(xln[hc][:, kvc * P:(kvc + 1) * P]),
                                _r(wv_sb[hc][:]),
                                start=(hc == 0), stop=(hc == HC - 1))
                        for kvh in range(4):
                            nc.scalar.copy(
                                v_sb[kvc][:, kvh * 65:kvh * 65 + 64],
                                ps[:, kvh * 64:(kvh + 1) * 64])
                            nc.any.memset(
                                v_sb[kvc][:, kvh * 65 + 64:kvh * 65 + 65], 1.0)

                # Q projection (own tokens), per head [64, R]
                qTh = [kvq.tile([64, R], f32, tag=f"qTh{i}", name=f"qTh{i}")
                       for i in range(NH)]
                with ExitStack() as ph:
                    wqp = ph.enter_context(tc.tile_pool(name="wqp", bufs=2))
                    psC = ph.enter_context(tc.tile_pool(name="psC", bufs=2, space="PSUM"))
                    for mp in range(4):
                        wq_sb = [wqp.tile([P, 256], f32, tag=f"wqt{i}", name=f"wqt{i}")
                                 for i in range(HC)]
                        for hc in range(HC):
                            nc.sync.dma_start(wq_sb[hc][:], wq[mp, hc])
                        for hh in range(4):
                            h = mp * 4 + hh
                            ps = psC.tile([64, R], f32, tag="psq")
                            for hc in range(HC):
                                nc.tensor.matmul(
                                    ps[:], _r(wq_sb[hc][:, hh * 64:(hh + 1) * 64]),
                                    _r(xlno[hc][:]),
                                    start=(hc == 0), stop=(hc == HC - 1))
                            nc.scalar.copy(qTh[h][:], ps[:])

            # -------- Phase D: attention --------
            attnh = [attp.tile([64, R], f32, tag=f"attnh{i}", name=f"attnh{i}")
                     for i in range(NH)]
            with ExitStack() as ph:
                psS = ph.enter_context(tc.tile_pool(name="psS", bufs=4, space="PSUM"))
                psO = ph.enter_context(tc.tile_pool(name="psO", bufs=3, space="PSUM"))
                psBC = ph.enter_context(tc.tile_pool(name="psBC", bufs=1, space="PSUM"))
                probp = ph.enter_context(tc.tile_pool(name="probp", bufs=4))
                smallp = ph.enter_context(tc.tile_pool(name="smallD", bufs=2))
                for h in range(NH):
                    hkv = h // (NH // NKV)
                    psum_o = psO.tile([65, R], f32, tag="psO")
                    for kvb in range(8):
                        q0 = _jmin(kvb) * P
                        j = kvb // 2
                        ps = psS.tile([P, R], f32, tag="psS")
                        nc.tensor.matmul(
                            ps[:, q0:R],
                            _r(kTh[hkv][:, kvb * P:(kvb + 1) * P]),
                            _r(qTh[h][:, q0:R]),
                            start=True, stop=True)
                        nc.vector.tensor_add(ps[:, j * P:(j + 1) * P],
                                             ps[:, j * P:(j + 1) * P],
                                             maskt_sb[:, kvb, :])
                        pr = probp.tile([P, R], f32, tag="probs")
                        nc.scalar.activation(pr[:, q0:R], ps[:, q0:R], AF.Exp)
                        nc.tensor.matmul(
                            psum_o[:, q0:R],
                            _r(v_sb[kvb][:, hkv * 65:(hkv + 1) * 65]),
                            _r(pr[:, q0:R]),
                            start=(kvb == 0), stop=(kvb == 7))
                    recips = smallp.tile([P, R], f32, tag="recips")
                    nc.scalar.copy(recips[64:65, :], psum_o[64:65, :])
                    nc.vector.reciprocal(recips[64:65, :], recips[64:65, :])
                    bc = psBC.tile([64, R], f32, tag="bc")
                    nc.tensor.matmul(bc[:], _r(ones_full[64:65, :]),
                                     _r(recips[64:65, :]), start=True, stop=True)
                    onorm = smallp.tile([64, R], f32, tag="onorm")
                    nc.scalar.copy(onorm[:], psum_o[0:64, :])
                    nc.vector.tensor_mul(attnh[h][:], onorm[:], bc[:])

            kvq_stack.close()  # free kTh/v_sb/qTh before the out projection

            # -------- Phase E: out projection + residual --------
            with ExitStack() as ph:
                wop = ph.enter_context(tc.tile_pool(name="wop", bufs=1))
                xop = ph.enter_context(tc.tile_pool(name="xo2", bufs=2))
                psE = ph.enter_context(tc.tile_pool(name="psE", bufs=2, space="PSUM"))
                wo_sb = [wop.tile([64, H], f32, tag=f"wo{h}", name=f"wo{h}")
                         for h in range(NH)]
                for h in range(NH):
                    nc.sync.dma_start(wo_sb[h][:], wo[h])
                for m in range(HC):
                    ps = psE.tile([P, R], f32, tag="pse")
                    for h in range(NH):
                        nc.tensor.matmul(
                            ps[:], _r(wo_sb[h][:, m * P:(m + 1) * P]),
                            _r(attnh[h][:]),
                            start=(h == 0), stop=(h == NH - 1))
                    xo = xop.tile([P, R], f32, tag="xo")
                    nc.sync.dma_start(xo[:], xot[m])
                    nc.vector.tensor_add(hown[m][:], ps[:], xo[:])

        # -------- Phase F: rmsnorm2 -> ag_in --------
        with ExitStack() as ph:
            sqp = ph.enter_context(tc.tile_pool(name="sq2p", bufs=2))
            psF = ph.enter_context(tc.tile_pool(name="psF", bufs=1, space="PSUM"))
            rowp = ph.enter_context(tc.tile_pool(name="rowF", bufs=1))
            hlnp = ph.enter_context(tc.tile_pool(name="hlnp", bufs=2))
            ss2 = psF.tile([1, R], f32, tag="ss2")
            for hc in range(HC):
                sq = sqp.tile([P, R], f32, tag="sq2")
                nc.vector.tensor_mul(sq[:], hown[hc][:], hown[hc][:])
                nc.tensor.matmul(ss2[:], _r(ones_col[:]), _r(sq[:]),
                                 start=(hc == 0), stop=(hc == HC - 1))
            inv2 = rowp.tile([1, R], f32, tag="inv2")
            nc.scalar.activation(inv2[:], ss2[:], AF.Sqrt, bias=eps_b[:],
                                 scale=1.0 / H)
            with nc.allow_low_precision(reason="fp32r scale rows"):
                nc.vector.reciprocal(inv2[:], inv2[:])
            inv2b = psF.tile([P, R], f32, tag="inv2b")
            nc.tensor.matmul(inv2b[:], _r(ones_row[:]), _r(inv2[:]),
                             start=True, stop=True)
            for hc in range(HC):
                hl = hlnp.tile([P, R], f32, tag="hln")
                nc.vector.tensor_mul(hl[:], hown[hc][:], inv2b[:])
                nc.sync.dma_start(ag_in[hc], hl[:])

        # -------- Phase G: AllGather --------
        nc.gpsimd.collective_compute(
            "AllGather", OP.bypass, replica_groups=rg,
            ins=[ag_in.opt()], outs=[ag_out.opt()])

        # -------- Phase H0: routing (all 8 blocks) --------
        cbs = [const.tile([P, R], f32, tag=f"cbs{tb}", name=f"cbs{tb}")
               for tb in range(8)]
        with ExitStack() as h0:
            hfp = h0.enter_context(tc.tile_pool(name="hlf", bufs=2))
            psL = h0.enter_context(tc.tile_pool(name="psL", bufs=2, space="PSUM"))
            smp = h0.enter_context(tc.tile_pool(name="smR", bufs=2))
            for tb in range(8):
                hlf = [hfp.tile([P, R], f32, tag=f"hlf{i}", name=f"hlf{i}")
                       for i in range(HC)]
                for hc in range(HC):
                    nc.sync.dma_start(hlf[hc][:], ag_out[tb, hc])
                cpack = smp.tile([P, 4], f32, tag="cpack")
                for ch in range(4):
                    psl = psL.tile([P, E], f32, tag="psl")
                    for hc in range(HC):
                        nc.tensor.matmul(
                            psl[:], hlf[hc][:, ch * P:(ch + 1) * P],
                            wgate_sb[:, hc, :],
                            start=(hc == 0), stop=(hc == HC - 1))
                    ex = smp.tile([P, E], f32, tag="ex")
                    s1 = smp.tile([P, 1], f32, tag="s1")
                    nc.scalar.activation(ex[:], psl[:], AF.Exp, accum_out=s1[:])
                    m1 = smp.tile([P, 1], f32, tag="m1")
                    nc.vector.reduce_max(m1[:], ex[:], AX.X)
                    t1 = smp.tile([P, E], f32, tag="t1")
                    nc.vector.tensor_scalar(t1[:], ex[:], m1[:], None, OP.is_ge)
                    msk = smp.tile([P, E], f32, tag="msk")
                    nc.vector.scalar_tensor_tensor(
                        msk[:], t1[:], -1e30, ex[:], op0=OP.mult, op1=OP.add)
                    m2 = smp.tile([P, 1], f32, tag="m2")
                    nc.vector.reduce_max(m2[:], msk[:], AX.X)
                    keep = smp.tile([P, 1], f32, tag="keep")
                    nc.vector.tensor_scalar(keep[:], ex[:, 0:1], m2[:], None,
                                            OP.is_ge)
                    cw = smp.tile([P, 1], f32, tag="cw")
                    nc.vector.tensor_mul(cw[:], ex[:, 0:1], keep[:])
                    rs1 = smp.tile([P, 1], f32, tag="rs1")
                    nc.vector.reciprocal(rs1[:], s1[:])
                    nc.vector.tensor_scalar(cpack[:, ch:ch + 1], cw[:], rs1[:],
                                            None, OP.mult)
                nc.sync.dma_start(c_dram[tb], cpack[:])
                for j in range(4):
                    bsrc = (c_dram[tb, :, j:j + 1]
                            .rearrange("t o -> o t").to_broadcast((P, P)))
                    nc.sync.dma_start(cbs[tb][:, j * P:(j + 1) * P], bsrc)

        # -------- Phase H1: expert compute (bf16, N=1024) --------
        with ExitStack() as h1:
            hlrp = h1.enter_context(tc.tile_pool(name="hlrp", bufs=2))
            for sb in range(4):
                with ExitStack() as ph:
                    hln_r = [hlrp.tile([P, 2 * R], bf16, tag=f"hlr{i}",
                                       name=f"hlr{i}") for i in range(HC)]
                    for hc in range(HC):
                        nc.gpsimd.dma_start(hln_r[hc][:, 0:R], ag_out[2 * sb, hc])
                        nc.gpsimd.dma_start(hln_r[hc][:, R:2 * R],
                                            ag_out[2 * sb + 1, hc])

                    prodp = ph.enter_context(tc.tile_pool(name="prodp", bufs=1))
                    prod = [prodp.tile([P, 2 * R], bf16, tag=f"pr{i}",
                                       name=f"pr{i}") for i in range(FC)]
                    with ExitStack() as gu:
                        wgp = gu.enter_context(tc.tile_pool(name="wgp", bufs=3))
                        psG = gu.enter_context(
                            tc.tile_pool(name="psG", bufs=1, space="PSUM"))
                        psU = gu.enter_context(
                            tc.tile_pool(name="psU", bufs=1, space="PSUM"))
                        gsp = gu.enter_context(tc.tile_pool(name="gsp", bufs=2))
                        for mp in range(FC // 2):
                            wg_sb = [wgp.tile([P, 256], bf16, tag=f"wgt{i}",
                                              name=f"wgt{i}") for i in range(HC)]
                            wu_sb = [wgp.tile([P, 256], bf16, tag=f"wut{i}",
                                              name=f"wut{i}") for i in range(HC)]
                            for hc in range(HC):
                                nc.sync.dma_start(wg_sb[hc][:], wg[mp, hc])
                                nc.sync.dma_start(wu_sb[hc][:], wu[mp, hc])
                            for half in range(2):
                                mf = mp * 2 + half
                                gps = psG.tile([P, 2 * R], f32, tag="gps")
                                ups = psU.tile([P, 2 * R], f32, tag="ups")
                                for hc in range(HC):
                                    for n in range(2):
                                        nc.tensor.matmul(
                                            gps[:, n * R:(n + 1) * R],
                                            wg_sb[hc][:, half * P:(half + 1) * P],
                                            hln_r[hc][:, n * R:(n + 1) * R],
                                            start=(hc == 0), stop=(hc == HC - 1))
                                for hc in range(HC):
                                    for n in range(2):
                                        nc.tensor.matmul(
                                            ups[:, n * R:(n + 1) * R],
                                            wu_sb[hc][:, half * P:(half + 1) * P],
                                            hln_r[hc][:, n * R:(n + 1) * R],
                                            start=(hc == 0), stop=(hc == HC - 1))
                                gs = gsp.tile([P, 2 * R], f32, tag="gs")
                                nc.scalar.activation(gs[:], gps[:], AF.Silu)
                                with nc.allow_low_precision(reason="bf16 experts"):
                                    nc.vector.tensor_mul(prod[mf][:], gs[:], ups[:])

                    with ExitStack() as dn:
                        wdp = dn.enter_context(tc.tile_pool(name="wdp", bufs=3))
                        psY = dn.enter_context(
                            tc.tile_pool(name="psY", bufs=2, space="PSUM"))
                        ysp = dn.enter_context(tc.tile_pool(name="ysp", bufs=2))
                        for mp in range(HC // 2):
                            wd_sb = [wdp.tile([P, 256], bf16, tag=f"wdt{i}",
                                              name=f"wdt{i}") for i in range(FC)]
                            for fc in range(FC):
                                nc.sync.dma_start(wd_sb[fc][:], wd[mp, fc])
                            for half in range(2):
                                mo = mp * 2 + half
                                yps = psY.tile([P, 2 * R], f32, tag="yps")
                                for fc in range(FC):
                                    for n in range(2):
                                        nc.tensor.matmul(
                                            yps[:, n * R:(n + 1) * R],
                                            wd_sb[fc][:, half * P:(half + 1) * P],
                                            prod[fc][:, n * R:(n + 1) * R],
                                            start=(fc == 0), stop=(fc == FC - 1))
                                ys = ysp.tile([P, 2 * R], bf16, tag="ys")
                                with nc.allow_low_precision(reason="bf16 rs"):
                                    for n in range(2):
                                        nc.vector.tensor_mul(
                                            ys[:, n * R:(n + 1) * R],
                                            yps[:, n * R:(n + 1) * R],
                                            cbs[2 * sb + n][:])
                                for n in range(2):
                                    nc.sync.dma_start(rs_in[2 * sb + n, mo],
                                                      ys[:, n * R:(n + 1) * R])

        # -------- Phase I: ReduceScatter --------
        nc.gpsimd.collective_compute(
            "ReduceScatter", OP.add, replica_groups=rg,
            ins=[rs_in.opt()], outs=[rs_out.opt()])

        # -------- Phase J: final residual + output --------
        with ExitStack() as ph:
            finp = ph.enter_context(tc.tile_pool(name="finp", bufs=2))
            for hc in range(HC):
                rsb = finp.tile([P, R], bf16, tag="rsb")
                nc.sync.dma_start(rsb[:], rs_out[hc])
                fin = finp.tile([P, R], f32, tag="fin")
                nc.vector.tensor_add(fin[:], hown[hc][:], rsb[:])
                nc.sync.dma_start(out[hc], fin[:])


# ---------------------------------------------------------------------------
# Host side
# ---------------------------------------------------------------------------

def _own_rows(c):
    hf = c % 2
    return np.concatenate(
        [np.arange((2 * j + hf) * P, (2 * j + hf + 1) * P) for j in range(4)])


def prep_inputs(inputs):
    x = np.ascontiguousarray(np.asarray(inputs["x"], np.float32))
    mask = np.asarray(inputs["attention_mask"], np.float32)[0, 0]
    wln1 = np.asarray(inputs["w_ln1"], np.float32)
    wln2 = np.asarray(inputs["w_ln2"], np.float32)
    Wq = np.asarray(inputs["Wq"], np.float32)
    Wk = np.asarray(inputs["Wk"], np.float32)
    Wv = np.asarray(inputs["Wv"], np.float32)
    Wo = np.asarray(inputs["Wo"], np.float32)
    Wgate = np.asarray(inputs["Wgate"], np.float32)
    Wg = np.asarray(inputs["Wg"], np.float32)
    Wu = np.asarray(inputs["Wu"], np.float32)
    Wd = np.asarray(inputs["Wd"], np.float32)

    c32 = lambda a: np.ascontiguousarray(a, np.float32)
    Wq_f = (wln1[:, None] * Wq) / np.sqrt(np.float32(HD))
    Wk_f = wln1[:, None] * Wk
    Wv_f = wln1[:, None] * Wv
    wq_t = c32(Wq_f.reshape(HC, P, 4, 256).transpose(2, 0, 1, 3))
    wk_t = c32(Wk_f.reshape(HC, P, 256))
    wv_t = c32(Wv_f.reshape(HC, P, 256))
    wo_t = c32(Wo.reshape(NH, HD, H))

    in_maps = []
    for c in range(N_CORES):
        b = c // 2
        rows = _own_rows(c)
        xT = x[b].T  # [H, S]
        xt_t = c32(xT.reshape(HC, P, S))
        xot_t = c32(xT[:, rows].reshape(HC, P, R))
        mt = np.empty((8, P, P), np.float32)
        for j in range(4):
            qrows = rows[j * P:(j + 1) * P]
            for w in range(2):
                kvcols = np.arange((2 * j + w) * P, (2 * j + w + 1) * P)
                mt[j * 2 + w] = mask[np.ix_(qrows, kvcols)].T
        perm = [c] + [e for e in range(E) if e != c]
        wgate_f = c32((wln2[:, None] * Wgate[:, perm]).reshape(HC, P, E))
        cbf = lambda a: np.ascontiguousarray(a.astype(ml_dtypes.bfloat16))
        wg_f = cbf((wln2[:, None] * Wg[c]).reshape(HC, P, FC // 2, 256)
                   .transpose(2, 0, 1, 3))
        wu_f = cbf((wln2[:, None] * Wu[c]).reshape(HC, P, FC // 2, 256)
                   .transpose(2, 0, 1, 3))
        wd_t = cbf(Wd[c].reshape(FC, P, HC // 2, 256).transpose(2, 0, 1, 3))
        in_maps.append({
            "xt": xt_t, "xot": xot_t, "maskt": c32(mt),
            "wq": wq_t, "wk": wk_t, "wv": wv_t, "wo": wo_t,
            "wgate": wgate_f, "wg": wg_f, "wu": wu_f, "wd": wd_t,
        })
    return in_maps


_NC_CACHE = None


def get_program():
    global _NC_CACHE
    if _NC_CACHE is None:
        _NC_CACHE = build_program()
    return _NC_CACHE


def run(in_maps, **kwargs):
    nc = get_program()
    return run_bass_kernel_spmd(nc, in_maps, list(range(N_CORES)), **kwargs)


def assemble(results):
    out = np.zeros((B * S, H), np.float32)
    for c in range(N_CORES):
        block = results[c]["out"].reshape(H, R).T  # [512 local rows, H]
        out[(c // 2) * S + _own_rows(c)] = block
    return out.reshape(B, S, H)


def kernel(**inputs):
    in_maps = prep_inputs(inputs)
    res = run(in_maps)
    return assemble(res.results)


# revision 17
# speedup vs baseline: 1.6496x; 1.6496x over previous
"""Trainium2 Bass kernel for nn_DecoderLayer (GQA attention + top-2-of-8 MoE).

Sharding (8 NeuronCores, SPMD single program):
  - Attention: data-parallel. Core c owns batch c//2 and an interleaved set of
    512 query rows (128-token blocks 2j + c%2, j=0..3) so the causal-length
    structure is identical on every core; per-core mask tiles carry the
    difference as data.
  - MoE: expert-parallel, dense. Core c owns expert c; an AllGather shares the
    post-attention normed activations, every core runs its expert over all
    4096 tokens scaled by the top-2 combine weight, and a ReduceScatter(add)
    returns each core its own tokens' sum over experts.
  - All activations are kept feature-major ([feature, token]) so matmul chains
    need no transposes; per-token scalars are broadcast across partitions with
    K=1 matmuls (or DMA broadcast from DRAM).
  - w_ln1/w_ln2, the 1/sqrt(HD) score scale, and the "own expert = column 0"
    Wgate permutation are folded into the weights on the host.
  - Matmuls run as float32r (FP22, 1-pass PE).

kernel(**inputs) takes the full unsharded inputs and returns the full output.
"""

import sys
from contextlib import ExitStack

if "/opt/trn_rl_repo" not in sys.path:
    sys.path.insert(0, "/opt/trn_rl_repo")

import ml_dtypes
import numpy as np

import concourse.bass as bass  # noqa: F401
import concourse.mybir as mybir
import concourse.tile as tile
from concourse import bacc
from concourse.bass_utils import run_bass_kernel_spmd
from concourse.masks import make_identity

P = 128
B, S, H = 4, 1024, 1024
NH, NKV, HD = 16, 4, 64
E, F = 8, 2048
EPS = 1e-6
N_CORES = 8
R = 512                      # tokens owned per core
HC = H // P                  # 8 feature chunks of H
FC = F // P                  # 16 feature chunks of F
f32 = mybir.dt.float32
f32r = mybir.dt.float32r
bf16 = mybir.dt.bfloat16
AF = mybir.ActivationFunctionType
OP = mybir.AluOpType
AX = mybir.AxisListType


def _r(ap):
    """Matmul operands are float32r-typed tiles already; identity."""
    return ap


def _jmin(kvb):
    # first local q-chunk whose kv extent (2j+2 blocks) includes kv block kvb
    return max(0, -(-(kvb - 1) // 2))


def build_program():
    nc = bacc.Bacc("TRN2", target_bir_lowering=False, debug=False,
                   num_devices=N_CORES)

    xt = nc.declare_dram_parameter("xt", [HC, P, S], f32, isOutput=False)
    xot = nc.declare_dram_parameter("xot", [HC, P, R], f32, isOutput=False)
    maskt = nc.declare_dram_parameter("maskt", [8, P, P], f32, isOutput=False)
    wq = nc.declare_dram_parameter("wq", [4, HC, P, 256], f32, isOutput=False)
    wk = nc.declare_dram_parameter("wk", [HC, P, 256], f32, isOutput=False)
    wv = nc.declare_dram_parameter("wv", [HC, P, 256], f32, isOutput=False)
    wo = nc.declare_dram_parameter("wo", [NH, HD, H], f32, isOutput=False)
    wgate = nc.declare_dram_parameter("wgate", [HC, P, E], f32, isOutput=False)
    wg = nc.declare_dram_parameter("wg", [FC // 2, HC, P, 256], bf16, isOutput=False)
    wu = nc.declare_dram_parameter("wu", [FC // 2, HC, P, 256], bf16, isOutput=False)
    wd = nc.declare_dram_parameter("wd", [HC // 2, FC, P, 256], bf16, isOutput=False)
    out = nc.declare_dram_parameter("out", [HC, P, R], f32, isOutput=True)

    with tile.TileContext(nc) as tc:
        _emit(nc, tc, xt=xt, xot=xot, maskt=maskt, wq=wq, wk=wk, wv=wv, wo=wo,
              wgate=wgate, wg=wg, wu=wu, wd=wd, out=out)
    nc.compile()
    return nc


def _emit(nc, tc, *, xt, xot, maskt, wq, wk, wv, wo, wgate, wg, wu, wd, out):
    rg = [list(range(N_CORES))]

    with ExitStack() as top:
        const = top.enter_context(tc.tile_pool(name="const", bufs=1))
        dram = top.enter_context(tc.tile_pool(name="dram", bufs=1, space="DRAM"))

        ones_col = const.tile([P, 1], f32, tag="ones_col")   # lhsT for partition sums
        ones_row = const.tile([1, P], f32, tag="ones_row")   # lhsT for broadcasts
        eps_b = const.tile([1, 1], f32, tag="eps_b")
        ones_full = const.tile([P, P], f32, tag="ones_full")
        nc.any.memset(ones_full[:], 1.0)
        ident = const.tile([P, P], f32, tag="ident")
        make_identity(nc, ident[:])
        nc.any.memset(ones_col[:], 1.0)
        nc.any.memset(ones_row[:], 1.0)
        nc.any.memset(eps_b[:], EPS)

        maskt_sb = const.tile([P, 8, P], f32, tag="maskt")
        nc.sync.dma_start(maskt_sb[:], maskt[:].rearrange("m p q -> p m q"))
        wgate_sb = const.tile([P, HC, E], f32, tag="wgate")
        nc.sync.dma_start(wgate_sb[:], wgate[:].rearrange("k p e -> p k e"))

        # DRAM scratch
        ag_in = dram.tile([HC, P, R], f32, tag="ag_in")
        ag_out = dram.tile([N_CORES, HC, P, R], f32, tag="ag_out", addr_space="Shared")
        rs_in = dram.tile([N_CORES, HC, P, R], bf16, tag="rs_in")
        rs_out = dram.tile([HC, P, R], bf16, tag="rs_out")

        hown = [const.tile([P, R], f32, tag=f"hown{m}", name=f"hown{m}")
                for m in range(HC)]

        with ExitStack() as attlife:
            attp = attlife.enter_context(tc.tile_pool(name="attp", bufs=1))
            kvq_stack = attlife.enter_context(ExitStack())
            kvq = kvq_stack.enter_context(tc.tile_pool(name="kvq", bufs=1))

            # -------- Phases A-C: x load, rmsnorm1, K/V/Q projections --------
            # rmsnorm1 is applied IN PLACE: after phase A, xt_sb/xot_sb hold
            # the normalized activations (xln / xlno).
            with ExitStack() as phA:
                xp = phA.enter_context(tc.tile_pool(name="xtp", bufs=1))
                xt_sb = [xp.tile([P, S], f32, tag=f"xt{i}", name=f"xt{i}")
                         for i in range(HC)]
                xot_sb = [xp.tile([P, R], f32, tag=f"xoA{i}", name=f"xoA{i}")
                          for i in range(HC)]
                xln, xlno = xt_sb, xot_sb

                with ExitStack() as ph:
                    sqp = ph.enter_context(tc.tile_pool(name="sqp", bufs=1))
                    psA = ph.enter_context(tc.tile_pool(name="psA", bufs=1, space="PSUM"))
                    rowp = ph.enter_context(tc.tile_pool(name="rowA", bufs=1))

                    ss_ps = [psA.tile([1, R], f32, tag=f"ss{n}", name=f"ss{n}")
                             for n in range(2)]
                    sso_ps = psA.tile([1, R], f32, tag="sso")
                    for hc in range(HC):
                        nc.sync.dma_start(xt_sb[hc][:], xt[hc])
                        nc.sync.dma_start(xot_sb[hc][:], xot[hc])
                    for hc in range(HC):
                        sq = sqp.tile([P, S], f32, tag="sq")
                        nc.vector.tensor_mul(sq[:], xt_sb[hc][:], xt_sb[hc][:])
                        for n in range(2):
                            nc.tensor.matmul(
                                ss_ps[n][:], _r(ones_col[:]),
                                _r(sq[:, n * R:(n + 1) * R]),
                                start=(hc == 0), stop=(hc == HC - 1))
                        sqo = sqp.tile([P, R], f32, tag="sqo")
                        nc.vector.tensor_mul(sqo[:], xot_sb[hc][:], xot_sb[hc][:])
                        nc.tensor.matmul(sso_ps[:], _r(ones_col[:]), _r(sqo[:]),
                                         start=(hc == 0), stop=(hc == HC - 1))

                    inv_row = rowp.tile([1, S], f32, tag="inv")
                    invo_row = rowp.tile([1, R], f32, tag="invo")
                    for n in range(2):
                        nc.scalar.activation(inv_row[:, n * R:(n + 1) * R],
                                             ss_ps[n][:], AF.Sqrt,
                                             bias=eps_b[:], scale=1.0 / H)
                    with nc.allow_low_precision(reason="fp32r scale rows"):
                        nc.vector.reciprocal(inv_row[:], inv_row[:])
                    nc.scalar.activation(invo_row[:], sso_ps[:], AF.Sqrt,
                                         bias=eps_b[:], scale=1.0 / H)
                    with nc.allow_low_precision(reason="fp32r scale rows"):
                        nc.vector.reciprocal(invo_row[:], invo_row[:])

                    invb_ps = [psA.tile([P, R], f32, tag=f"invb{n}", name=f"invb{n}")
                               for n in range(2)]
                    invob_ps = psA.tile([P, R], f32, tag="invob")
                    for n in range(2):
                        nc.tensor.matmul(invb_ps[n][:], _r(ones_row[:]),
                                         _r(inv_row[:, n * R:(n + 1) * R]),
                                         start=True, stop=True)
                    nc.tensor.matmul(invob_ps[:], _r(ones_row[:]), _r(invo_row[:]),
                                     start=True, stop=True)
                    for hc in range(HC):
                        for n in range(2):
                            nc.vector.tensor_mul(xln[hc][:, n * R:(n + 1) * R],
                                                 xt_sb[hc][:, n * R:(n + 1) * R],
                                                 invb_ps[n][:])
                        nc.vector.tensor_mul(xlno[hc][:], xot_sb[hc][:],
                                             invob_ps[:])

                # K (per kv-head, [64, S]) and V (token-major, [128, 256])
                kTh = [kvq.tile([64, S], f32r, tag=f"kTh{i}", name=f"kTh{i}")
                       for i in range(NKV)]
                v_sb = [kvq.tile([P, 4 * 65], f32r, tag=f"v{i}", name=f"v{i}")
                        for i in range(HC)]
                with ExitStack() as ph:
                    wp = ph.enter_context(tc.tile_pool(name="wkv", bufs=1))
                    psB = ph.enter_context(tc.tile_pool(name="psB", bufs=2, space="PSUM"))
                    wk_sb = [wp.tile([P, 256], f32, tag=f"wk{i}", name=f"wk{i}")
                             for i in range(HC)]
                    wv_sb = [wp.tile([P, 256], f32, tag=f"wv{i}", name=f"wv{i}")
                             for i in range(HC)]
                    for hc in range(HC):
                        nc.sync.dma_start(wk_sb[hc][:], wk[hc])
                        nc.sync.dma_start(wv_sb[hc][:], wv[hc])
                    for hkv in range(NKV):
                        for n in range(2):
                            ps = psB.tile([64, R], f32, tag="psk")
                            for hc in range(HC):
                                nc.tensor.matmul(
                                    ps[:],
                                    _r(wk_sb[hc][:, hkv * 64:(hkv + 1) * 64]),
                                    _r(xln[hc][:, n * R:(n + 1) * R]),
                                    start=(hc == 0), stop=(hc == HC - 1))
                            nc.scalar.copy(kTh[hkv][:, n * R:(n + 1) * R], ps[:])
                    for kvc in range(HC):
                        ps = psB.tile([P, 256], f32, tag="psv")
                        for hc in range(HC):
                            nc.tensor.matmul(
                                ps[:], _r(xln[hc][:, kvc * P:(kvc + 1) * P]),
                                _r(wv_sb[hc][:]),
                                start=(hc == 0), stop=(hc == HC - 1))
                        for kvh in range(4):
                            nc.scalar.copy(
                                v_sb[kvc][:, kvh * 65:kvh * 65 + 64],
                                ps[:, kvh * 64:(kvh + 1) * 64])
                            nc.any.memset(
                                v_sb[kvc][:, kvh * 65 + 64:kvh * 65 + 65]
                                .bitcast(f32), 1.0)

                # Q projection (own tokens), per head [64, R]
                qTh = [kvq.tile([64, R], f32r, tag=f"qTh{i}", name=f"qTh{i}")
                       for i in range(NH)]
                with ExitStack() as ph:
                    wqp = ph.enter_context(tc.tile_pool(name="wqp", bufs=2))
                    psC = ph.enter_context(tc.tile_pool(name="psC", bufs=2, space="PSUM"))
                    for mp in range(4):
                        wq_sb = [wqp.tile([P, 256], f32, tag=f"wqt{i}", name=f"wqt{i}")
                                 for i in range(HC)]
                        for hc in range(HC):
                            nc.sync.dma_start(wq_sb[hc][:], wq[mp, hc])
                        for hh in range(4):
                            h = mp * 4 + hh
                            ps = psC.tile([64, R], f32, tag="psq")
                            for hc in range(HC):
                                nc.tensor.matmul(
                                    ps[:], _r(wq_sb[hc][:, hh * 64:(hh + 1) * 64]),
                                    _r(xlno[hc][:]),
                                    start=(hc == 0), stop=(hc == HC - 1))
                            nc.scalar.copy(qTh[h][:], ps[:])

            # -------- Phase D: attention --------
            attnh = [attp.tile([64, R], f32, tag=f"attnh{i}", name=f"attnh{i}")
                     for i in range(NH)]
            with ExitStack() as ph:
                psS = ph.enter_context(tc.tile_pool(name="psS", bufs=4, space="PSUM"))
                psO = ph.enter_context(tc.tile_pool(name="psO", bufs=3, space="PSUM"))
                psBC = ph.enter_context(tc.tile_pool(name="psBC", bufs=1, space="PSUM"))
                probp = ph.enter_context(tc.tile_pool(name="probp", bufs=4))
                smallp = ph.enter_context(tc.tile_pool(name="smallD", bufs=2))
                for h in range(NH):
                    hkv = h // (NH // NKV)
                    psum_o = psO.tile([65, R], f32, tag="psO")
                    for kvb in range(8):
                        q0 = _jmin(kvb) * P
                        j = kvb // 2
                        ps = psS.tile([P, R], f32, tag="psS")
                        nc.tensor.matmul(
                            ps[:, q0:R],
                            _r(kTh[hkv][:, kvb * P:(kvb + 1) * P]),
                            _r(qTh[h][:, q0:R]),
                            start=True, stop=True)
                        nc.vector.tensor_add(ps[:, j * P:(j + 1) * P],
                                             ps[:, j * P:(j + 1) * P],
                                             maskt_sb[:, kvb, :])
                        pr = probp.tile([P, R], f32r, tag="probs")
                        nc.scalar.activation(pr[:, q0:R], ps[:, q0:R], AF.Exp)
                        nc.tensor.matmul(
                            psum_o[:, q0:R],
                            _r(v_sb[kvb][:, hkv * 65:(hkv + 1) * 65]),
                            _r(pr[:, q0:R]),
                            start=(kvb == 0), stop=(kvb == 7))
                    recips = smallp.tile([P, R], f32, tag="recips")
                    nc.scalar.copy(recips[64:65, :], psum_o[64:65, :])
                    nc.vector.reciprocal(recips[64:65, :], recips[64:65, :])
                    bc = psBC.tile([64, R], f32, tag="bc")
                    nc.tensor.matmul(bc[:], _r(ones_full[64:65, 0:64]),
                                     _r(recips[64:65, :]), start=True, stop=True)
                    onorm = smallp.tile([64, R], f32, tag="onorm")
                    nc.scalar.copy(onorm[:], psum_o[0:64, :])
                    nc.vector.tensor_mul(attnh[h][:], onorm[:], bc[:])

            kvq_stack.close()  # free kTh/v_sb/qTh before the out projection

            # -------- Phase E: out projection + residual --------
            with ExitStack() as ph:
                wop = ph.enter_context(tc.tile_pool(name="wop", bufs=1))
                xop = ph.enter_context(tc.tile_pool(name="xo2", bufs=2))
                psE = ph.enter_context(tc.tile_pool(name="psE", bufs=2, space="PSUM"))
                wo_sb = [wop.tile([64, H], f32, tag=f"wo{h}", name=f"wo{h}")
                         for h in range(NH)]
                for h in range(NH):
                    nc.sync.dma_start(wo_sb[h][:], wo[h])
                for m in range(HC):
                    ps = psE.tile([P, R], f32, tag="pse")
                    for h in range(NH):
                        nc.tensor.matmul(
                            ps[:], _r(wo_sb[h][:, m * P:(m + 1) * P]),
                            _r(attnh[h][:]),
                            start=(h == 0), stop=(h == NH - 1))
                    xo = xop.tile([P, R], f32, tag="xo")
                    nc.sync.dma_start(xo[:], xot[m])
                    nc.vector.tensor_add(hown[m][:], ps[:], xo[:])

        # -------- Phase F: rmsnorm2 -> ag_in --------
        with ExitStack() as ph:
            sqp = ph.enter_context(tc.tile_pool(name="sq2p", bufs=2))
            psF = ph.enter_context(tc.tile_pool(name="psF", bufs=1, space="PSUM"))
            rowp = ph.enter_context(tc.tile_pool(name="rowF", bufs=1))
            hlnp = ph.enter_context(tc.tile_pool(name="hlnp", bufs=2))
            ss2 = psF.tile([1, R], f32, tag="ss2")
            for hc in range(HC):
                sq = sqp.tile([P, R], f32, tag="sq2")
                nc.vector.tensor_mul(sq[:], hown[hc][:], hown[hc][:])
                nc.tensor.matmul(ss2[:], _r(ones_col[:]), _r(sq[:]),
                                 start=(hc == 0), stop=(hc == HC - 1))
            inv2 = rowp.tile([1, R], f32, tag="inv2")
            nc.scalar.activation(inv2[:], ss2[:], AF.Sqrt, bias=eps_b[:],
                                 scale=1.0 / H)
            with nc.allow_low_precision(reason="fp32r scale rows"):
                nc.vector.reciprocal(inv2[:], inv2[:])
            inv2b = psF.tile([P, R], f32, tag="inv2b")
            nc.tensor.matmul(inv2b[:], _r(ones_row[:]), _r(inv2[:]),
                             start=True, stop=True)
            for hc in range(HC):
                hl = hlnp.tile([P, R], f32, tag="hln")
                nc.vector.tensor_mul(hl[:], hown[hc][:], inv2b[:])
                nc.sync.dma_start(ag_in[hc], hl[:])

        # -------- Phase G: AllGather --------
        nc.gpsimd.collective_compute(
            "AllGather", OP.bypass, replica_groups=rg,
            ins=[ag_in.opt()], outs=[ag_out.opt()])

        # -------- Phase H0: routing (all 8 blocks) --------
        cbs = [const.tile([P, R], f32, tag=f"cbs{tb}", name=f"cbs{tb}")
               for tb in range(8)]
        with ExitStack() as h0:
            hfp = h0.enter_context(tc.tile_pool(name="hlf", bufs=2))
            psL = h0.enter_context(tc.tile_pool(name="psL", bufs=2, space="PSUM"))
            psT = h0.enter_context(tc.tile_pool(name="psT", bufs=2, space="PSUM"))
            psB2 = h0.enter_context(tc.tile_pool(name="psB2", bufs=2, space="PSUM"))
            smp = h0.enter_context(tc.tile_pool(name="smR", bufs=2))
            for tb in range(8):
                hlf = [hfp.tile([P, R], f32, tag=f"hlf{i}", name=f"hlf{i}")
                       for i in range(HC)]
                for hc in range(HC):
                    nc.sync.dma_start(hlf[hc][:], ag_out[tb, hc])
                cpack = smp.tile([P, 4], f32, tag="cpack")
                for ch in range(4):
                    psl = psL.tile([P, E], f32, tag="psl")
                    for hc in range(HC):
                        nc.tensor.matmul(
                            psl[:], hlf[hc][:, ch * P:(ch + 1) * P],
                            wgate_sb[:, hc, :],
                            start=(hc == 0), stop=(hc == HC - 1))
                    ex = smp.tile([P, E], f32, tag="ex")
                    s1 = smp.tile([P, 1], f32, tag="s1")
                    nc.scalar.activation(ex[:], psl[:], AF.Exp, accum_out=s1[:])
                    m1 = smp.tile([P, 1], f32, tag="m1")
                    nc.vector.reduce_max(m1[:], ex[:], AX.X)
                    t1 = smp.tile([P, E], f32, tag="t1")
                    nc.vector.tensor_scalar(t1[:], ex[:], m1[:], None, OP.is_ge)
                    msk = smp.tile([P, E], f32, tag="msk")
                    nc.vector.scalar_tensor_tensor(
                        msk[:], t1[:], -1e30, ex[:], op0=OP.mult, op1=OP.add)
                    m2 = smp.tile([P, 1], f32, tag="m2")
                    nc.vector.reduce_max(m2[:], msk[:], AX.X)
                    keep = smp.tile([P, 1], f32, tag="keep")
                    nc.vector.tensor_scalar(keep[:], ex[:, 0:1], m2[:], None,
                                            OP.is_ge)
                    cw = smp.tile([P, 1], f32, tag="cw")
                    nc.vector.tensor_mul(cw[:], ex[:, 0:1], keep[:])
                    rs1 = smp.tile([P, 1], f32, tag="rs1")
                    nc.vector.reciprocal(rs1[:], s1[:])
                    nc.vector.tensor_scalar(cpack[:, ch:ch + 1], cw[:], rs1[:],
                                            None, OP.mult)
                cbp = psB2.tile([P, R], f32, tag="cbp")
                for j in range(4):
                    pst = psT.tile([1, P], f32, tag="pst")
                    nc.tensor.transpose(pst[:], cpack[:, j:j + 1], ident[:])
                    cTj = smp.tile([1, P], f32, tag="cTj")
                    nc.scalar.copy(cTj[:], pst[:])
                    nc.tensor.matmul(cbp[:, j * P:(j + 1) * P],
                                     ones_full[0:1, :], cTj[:],
                                     start=True, stop=True)
                nc.scalar.copy(cbs[tb][:], cbp[:])

        # -------- Phase H1: expert compute (bf16, N=1024) --------
        with ExitStack() as h1:
            hlrp = h1.enter_context(tc.tile_pool(name="hlrp", bufs=2))
            for sb in range(4):
                with ExitStack() as ph:
                    hln_r = [hlrp.tile([P, 2 * R], bf16, tag=f"hlr{i}",
                                       name=f"hlr{i}") for i in range(HC)]
                    for hc in range(HC):
                        nc.gpsimd.dma_start(hln_r[hc][:, 0:R], ag_out[2 * sb, hc])
                        nc.gpsimd.dma_start(hln_r[hc][:, R:2 * R],
                                            ag_out[2 * sb + 1, hc])

                    prodp = ph.enter_context(tc.tile_pool(name="prodp", bufs=1))
                    prod = [prodp.tile([P, 2 * R], bf16, tag=f"pr{i}",
                                       name=f"pr{i}") for i in range(FC)]
                    with ExitStack() as gu:
                        wgp = gu.enter_context(tc.tile_pool(name="wgp", bufs=3))
                        psG = gu.enter_context(
                            tc.tile_pool(name="psG", bufs=1, space="PSUM"))
                        psU = gu.enter_context(
                            tc.tile_pool(name="psU", bufs=1, space="PSUM"))
                        gsp = gu.enter_context(tc.tile_pool(name="gsp", bufs=2))
                        for mp in range(FC // 2):
                            wg_sb = [wgp.tile([P, 256], bf16, tag=f"wgt{i}",
                                              name=f"wgt{i}") for i in range(HC)]
                            wu_sb = [wgp.tile([P, 256], bf16, tag=f"wut{i}",
                                              name=f"wut{i}") for i in range(HC)]
                            for hc in range(HC):
                                nc.sync.dma_start(wg_sb[hc][:], wg[mp, hc])
                                nc.sync.dma_start(wu_sb[hc][:], wu[mp, hc])
                            for half in range(2):
                                mf = mp * 2 + half
                                gps = psG.tile([P, 2 * R], f32, tag="gps")
                                ups = psU.tile([P, 2 * R], f32, tag="ups")
                                for hc in range(HC):
                                    for n in range(2):
                                        nc.tensor.matmul(
                                            gps[:, n * R:(n + 1) * R],
                                            wg_sb[hc][:, half * P:(half + 1) * P],
                                            hln_r[hc][:, n * R:(n + 1) * R],
                                            start=(hc == 0), stop=(hc == HC - 1))
                                for hc in range(HC):
                                    for n in range(2):
                                        nc.tensor.matmul(
                                            ups[:, n * R:(n + 1) * R],
                                            wu_sb[hc][:, half * P:(half + 1) * P],
                                            hln_r[hc][:, n * R:(n + 1) * R],
                                            start=(hc == 0), stop=(hc == HC - 1))
                                gs = gsp.tile([P, 2 * R], f32, tag="gs")
                                nc.scalar.activation(gs[:], gps[:], AF.Silu)
                                with nc.allow_low_precision(reason="bf16 experts"):
                                    nc.vector.tensor_mul(prod[mf][:], gs[:], ups[:])

                    with ExitStack() as dn:
                        wdp = dn.enter_context(tc.tile_pool(name="wdp", bufs=3))
                        psY = dn.enter_context(
                            tc.tile_pool(name="psY", bufs=2, space="PSUM"))
                        ysp = dn.enter_context(tc.tile_pool(name="ysp", bufs=2))
                        for mp in range(HC // 2):
                            wd_sb = [wdp.tile([P, 256], bf16, tag=f"wdt{i}",
                                              name=f"wdt{i}") for i in range(FC)]
                            for fc in range(FC):
                                nc.sync.dma_start(wd_sb[fc][:], wd[mp, fc])
                            for half in range(2):
                                mo = mp * 2 + half
                                yps = psY.tile([P, 2 * R], f32, tag="yps")
                                for fc in range(FC):
                                    for n in range(2):
                                        nc.tensor.matmul(
                                            yps[:, n * R:(n + 1) * R],
                                            wd_sb[fc][:, half * P:(half + 1) * P],
                                            prod[fc][:, n * R:(n + 1) * R],
                                            start=(fc == 0), stop=(fc == FC - 1))
                                ys = ysp.tile([P, 2 * R], bf16, tag="ys")
                                with nc.allow_low_precision(reason="bf16 rs"):
                                    for n in range(2):
                                        nc.vector.tensor_mul(
                                            ys[:, n * R:(n + 1) * R],
                                            yps[:, n * R:(n + 1) * R],
                                            cbs[2 * sb + n][:])
                                for n in range(2):
                                    nc.sync.dma_start(rs_in[2 * sb + n, mo],
                                                      ys[:, n * R:(n + 1) * R])

        # -------- Phase I: ReduceScatter --------
        nc.gpsimd.collective_compute(
            "ReduceScatter", OP.add, replica_groups=rg,
            ins=[rs_in.opt()], outs=[rs_out.opt()])

        # -------- Phase J: final residual + output --------
        with ExitStack() as ph:
            finp = ph.enter_context(tc.tile_pool(name="finp", bufs=2))
            for hc in range(HC):
                rsb = finp.tile([P, R], bf16, tag="rsb")
                nc.sync.dma_start(rsb[:], rs_out[hc])
                fin = finp.tile([P, R], f32, tag="fin")
                nc.vector.tensor_add(fin[:], hown[hc][:], rsb[:])
                nc.sync.dma_start(out[hc], fin[:])


# ---------------------------------------------------------------------------
# Host side
# ---------------------------------------------------------------------------

def _own_rows(c):
    hf = c % 2
    return np.concatenate(
        [np.arange((2 * j + hf) * P, (2 * j + hf + 1) * P) for j in range(4)])


def prep_inputs(inputs):
    x = np.ascontiguousarray(np.asarray(inputs["x"], np.float32))
    mask = np.asarray(inputs["attention_mask"], np.float32)[0, 0]
    wln1 = np.asarray(inputs["w_ln1"], np.float32)
    wln2 = np.asarray(inputs["w_ln2"], np.float32)
    Wq = np.asarray(inputs["Wq"], np.float32)
    Wk = np.asarray(inputs["Wk"], np.float32)
    Wv = np.asarray(inputs["Wv"], np.float32)
    Wo = np.asarray(inputs["Wo"], np.float32)
    Wgate = np.asarray(inputs["Wgate"], np.float32)
    Wg = np.asarray(inputs["Wg"], np.float32)
    Wu = np.asarray(inputs["Wu"], np.float32)
    Wd = np.asarray(inputs["Wd"], np.float32)

    c32 = lambda a: np.ascontiguousarray(a, np.float32)
    Wq_f = (wln1[:, None] * Wq) / np.sqrt(np.float32(HD))
    Wk_f = wln1[:, None] * Wk
    Wv_f = wln1[:, None] * Wv
    wq_t = c32(Wq_f.reshape(HC, P, 4, 256).transpose(2, 0, 1, 3))
    wk_t = c32(Wk_f.reshape(HC, P, 256))
    wv_t = c32(Wv_f.reshape(HC, P, 256))
    wo_t = c32(Wo.reshape(NH, HD, H))

    in_maps = []
    for c in range(N_CORES):
        b = c // 2
        rows = _own_rows(c)
        xT = x[b].T  # [H, S]
        xt_t = c32(xT.reshape(HC, P, S))
        xot_t = c32(xT[:, rows].reshape(HC, P, R))
        mt = np.empty((8, P, P), np.float32)
        for j in range(4):
            qrows = rows[j * P:(j + 1) * P]
            for w in range(2):
                kvcols = np.arange((2 * j + w) * P, (2 * j + w + 1) * P)
                mt[j * 2 + w] = mask[np.ix_(qrows, kvcols)].T
        perm = [c] + [e for e in range(E) if e != c]
        wgate_f = c32((wln2[:, None] * Wgate[:, perm]).reshape(HC, P, E))
        cbf = lambda a: np.ascontiguousarray(a.astype(ml_dtypes.bfloat16))
        wg_f = cbf((wln2[:, None] * Wg[c]).reshape(HC, P, FC // 2, 256)
                   .transpose(2, 0, 1, 3))
        wu_f = cbf((wln2[:, None] * Wu[c]).reshape(HC, P, FC // 2, 256)
                   .transpose(2, 0, 1, 3))
        wd_t = cbf(Wd[c].reshape(FC, P, HC // 2, 256).transpose(2, 0, 1, 3))
        in_maps.append({
            "xt": xt_t, "xot": xot_t, "maskt": c32(mt),
            "wq": wq_t, "wk": wk_t, "wv": wv_t, "wo": wo_t,
            "wgate": wgate_f, "wg": wg_f, "wu": wu_f, "wd": wd_t,
        })
    return in_maps


_NC_CACHE = None


def get_program():
    global _NC_CACHE
    if _NC_CACHE is None:
        _NC_CACHE = build_program()
    return _NC_CACHE


def run(in_maps, **kwargs):
    nc = get_program()
    return run_bass_kernel_spmd(nc, in_maps, list(range(N_CORES)), **kwargs)


def assemble(results):
    out = np.zeros((B * S, H), np.float32)
    for c in range(N_CORES):
        block = results[c]["out"].reshape(H, R).T  # [512 local rows, H]
        out[(c // 2) * S + _own_rows(c)] = block
    return out.reshape(B, S, H)


def kernel(**inputs):
    in_maps = prep_inputs(inputs)
    res = run(in_maps)
    return assemble(res.results)
